# revision 28
# baseline (speedup 1.0000x reference)
"""Bahdanau temporal attention on 8 Trainium2 NeuronCores.

Full-input contract: kernel(**inputs) takes the unsharded numpy arrays
(query (32,1024), keys (32,4096,1024), Wq (1024,512), Wk (1024,512),
v (512,)) and returns the full output (32,1,1024) float32.

Sharding: data-parallel over batch. Each of the 8 cores processes 4
batches; Wq/Wk/v are replicated. No collectives.

Per-core algorithm (B_loc=4, S=4096, H=1024, A=512):
  q_t      = query @ Wq                          (B_loc, A)
  k_t      = keys @ Wk                           (B_loc, S, A)
  energy   = v . tanh(q_t + k_t)                 (B_loc, S)
  w        = exp(energy)         (unnormalized; |energy| <= |v|_1 so no
                                  max-subtraction is needed in fp32)
  ctx      = (w @ keys) / sum(w)                 (B_loc, H)

Single pass over keys. Per 512-row S-tile:
  - DMA keys tile (f32), DVE-cast to bf16,
  - xbar DMA-transpose bf16 -> keysT (contraction dim on partitions),
  - PE: k_t^T = Wk^T @ keys^T (bf16, f32 accum in PSUM),
  - ACT: T = tanh(k_t^T + q_t^T) with q_t as per-partition bias,
  - PE: energy = v^T @ T (fp32r),
  - ACT: w = exp(energy) with fused running-sum (accum_out),
  - PE transpose w to partitions; PE: ctx += w^T.T @ keys_nat (fp32r,
    full-f32 keys for accuracy).
"""

import sys

if "/opt/trn_rl_repo" not in sys.path:
    sys.path.insert(0, "/opt/trn_rl_repo")

import numpy as np

import concourse.bass as bass
import concourse.tile as tile
from concourse import bacc
from concourse import mybir
from concourse.bass_utils import run_bass_kernel_spmd
from concourse.masks import make_identity

F32 = mybir.dt.float32
F32R = mybir.dt.float32r
BF16 = mybir.dt.bfloat16

N_CORES = 8
B, S, H, A = 32, 4096, 1024, 512
B_LOC = B // N_CORES          # 4 batches per core
ST = 512                      # S-tile rows
N_ST = S // ST                # 8 S-tiles per batch
P = 128                       # partitions
HC = H // P                   # 8 contraction chunks
AC = A // P                   # 4 a-chunks
SC = ST // P                  # 4 s-chunks per S-tile


def build_bass():
    nc = bacc.Bacc()

    d_query = nc.declare_dram_parameter("query", [B_LOC, H], F32, isOutput=False)
    d_keys = nc.declare_dram_parameter("keys", [B_LOC, S, H], F32, isOutput=False)
    d_wq = nc.declare_dram_parameter("Wq", [H, A], F32, isOutput=False)
    d_wk = nc.declare_dram_parameter("Wk", [H, A], F32, isOutput=False)
    d_v = nc.declare_dram_parameter("v", [A], F32, isOutput=False)
    d_out = nc.declare_dram_parameter("out", [B_LOC, H], F32, isOutput=True)

    from contextlib import ExitStack

    with tile.TileContext(nc) as tc, ExitStack() as ctx:
        build_kernel_body(tc, d_query, d_keys, d_wq, d_wk, d_v, d_out, ctx)
    nc.compile()
    return nc


def build_kernel_body(tc, d_query, d_keys, d_wq, d_wk, d_v, d_out, ctx):
    nc = tc.nc

    consts = ctx.enter_context(tc.tile_pool(name="consts", bufs=1))
    keybf = ctx.enter_context(tc.tile_pool(name="keybf", bufs=6))
    keytp = ctx.enter_context(tc.tile_pool(name="keytp", bufs=6))
    tp = ctx.enter_context(tc.tile_pool(name="tp", bufs=3))
    smalls = ctx.enter_context(tc.tile_pool(name="smalls", bufs=2))
    pp_kt = ctx.enter_context(tc.tile_pool(name="pp_kt", bufs=3, space="PSUM"))
    pp_e = ctx.enter_context(tc.tile_pool(name="pp_e", bufs=2, space="PSUM"))
    pp_ctx = ctx.enter_context(tc.tile_pool(name="pp_ctx", bufs=3, space="PSUM"))

    # ---- constants ----
    # Wk in bf16, laid out [h' (part), hc, a]
    wk_bf = consts.tile([P, HC, A], BF16)
    nc.gpsimd.dma_start(
        out=wk_bf, in_=d_wq_rearr(d_wk)
    )  # SWDGE casts f32 -> bf16 in flight
    # Wq in bf16, same layout
    wq_sb = consts.tile([P, HC, A], BF16)
    nc.gpsimd.dma_start(out=wq_sb, in_=d_wq_rearr(d_wq))

    # v: load f32, DVE-cast into row 0 of a 16-row tile (single-producer
    # funnel so the xbar transpose carries only one wait), then xbar.
    v_f32 = consts.tile([1, A], F32)
    nc.gpsimd.dma_start(out=v_f32, in_=d_v[None, :])
    v16 = consts.tile([16, A], BF16)
    nc.vector.memset(v16, 0.0)
    nc.vector.tensor_copy(v16[0:1, :], v_f32)
    vT16 = consts.tile([P, AC, 16], BF16)
    nc.sync.dma_start(out=vT16, in_=v16, transpose=True)

    # query: same funnel pattern
    q_f32 = consts.tile([B_LOC, H], F32)
    nc.gpsimd.dma_start(out=q_f32, in_=d_query[:, :])
    q16 = consts.tile([16, H], BF16)
    nc.vector.memset(q16, 0.0)
    nc.vector.tensor_copy(q16[0:B_LOC, :], q_f32)
    qT16 = consts.tile([P, HC, 16], BF16)
    nc.sync.dma_start(out=qT16, in_=q16, transpose=True)

    # q_t = query @ Wq : psum (16, A), accumulate over hc
    ps_qt = pp_e.tile([16, A], F32, tag="pe")
    for hc in range(HC):
        nc.tensor.matmul(
            ps_qt,
            lhsT=qT16[:, hc, :],
            rhs=wq_sb[:, hc, :],
            start=(hc == 0),
            stop=(hc == HC - 1),
        )
    qt16 = consts.tile([16, A], BF16)
    nc.vector.memset(qt16, 0.0)
    nc.vector.tensor_copy(qt16[0:B_LOC, :], ps_qt[0:B_LOC, :])
    # xbar -> qtT16 (128, AC, 16); tanh bias per (ac, b) = qtT16[:, ac, b]
    qtT16 = consts.tile([P, AC, 16], BF16)
    nc.sync.dma_start(out=qtT16, in_=qt16, transpose=True)

    ones_bf = consts.tile([P, 1], BF16)
    nc.vector.memset(ones_bf, 1.0)

    # ---- main loop (2-stage pipelined emission: front i, compute i-1) ----
    iters = [(b, st) for b in range(B_LOC) for st in range(N_ST)]
    ctx_psums = {}
    front = {}
    front_loads = {}

    def stage_load(b, st):
        # load keys tile natural [s' (part), r, h], f32 -> bf16 in the DMA
        keys_bf = keybf.tile([P, SC, H], BF16, tag="kbf")
        nc.gpsimd.dma_start(
            out=keys_bf,
            in_=d_keys[b, st * ST : (st + 1) * ST, :].rearrange(
                "(p r) h -> p r h", p=P
            ),
        )
        return keys_bf

    def stage_xpose(b, st):
        keys_bf = front_loads[(b, st)]
        # transpose: keysT [h' (part), sc, hc, s']
        keysT = keytp.tile([P, SC, HC, P], BF16, tag="kT")
        for j in range(2):
            nc.sync.dma_start(
                out=keysT[:, 2 * j : 2 * j + 2, :, :],
                in_=keys_bf[:, 2 * j : 2 * j + 2, :],
                transpose=True,
            )
        return keys_bf, keysT

    def stage_compute(b, st):
        keys_bf, keysT = front.pop((b, st))

        first = st == 0
        last = st == N_ST - 1
        if first:
            ps_c0_new = pp_ctx.tile([1, 512], F32, tag="ctx")
            ps_c1_new = pp_ctx.tile([1, 512], F32, tag="ctx")
            ps_z_new = pp_ctx.tile([1, 1], F32, tag="ctx")
            ctx_psums[b] = (ps_c0_new, ps_c1_new, ps_z_new)
        ps_c0, ps_c1, _ = ctx_psums[b]

        # projection + tanh: T[a' (part), ac, s]
        T_sb = tp.tile([P, AC, ST], BF16, tag="T")
        for ac in range(AC):
            ps_kt = pp_kt.tile([P, ST], F32, tag="kt")
            for hc in range(HC):
                nc.tensor.matmul(
                    ps_kt,
                    lhsT=wk_bf[:, hc, ac * P : (ac + 1) * P],
                    rhs=keysT[:, :, hc, :],
                    start=(hc == 0),
                    stop=(hc == HC - 1),
                )
            nc.scalar.activation(
                T_sb[:, ac, :],
                ps_kt,
                mybir.ActivationFunctionType.Tanh,
                bias=qtT16[:, ac, b : b + 1],
            )

        # energy transposed: eT (128, SC) via regular matmuls (M=s chunk)
        ps_eT = pp_e.tile([P, SC], F32, tag="pe")
        for sc in range(SC):
            for ac in range(AC):
                nc.tensor.matmul(
                    ps_eT[:, sc : sc + 1],
                    lhsT=T_sb[:, ac, sc * P : (sc + 1) * P],
                    rhs=vT16[:, ac, 0:1],
                    start=(ac == 0),
                    stop=(ac == AC - 1),
                )

        # w^T = exp(eT) straight into SBUF, already s-on-partitions
        wT_sb = smalls.tile([P, SC], BF16, tag="wT")
        nc.scalar.activation(
            wT_sb,
            ps_eT,
            mybir.ActivationFunctionType.Exp,
        )

        # context accumulation: ctx (1, H) += w^T.T @ keys_bf
        # plus Z accumulation with a ones column (same bf16 weights as ctx)
        ps_z = ctx_psums[b][2]
        for sc in range(SC):
            nc.tensor.matmul(
                ps_c0,
                lhsT=wT_sb[:, sc : sc + 1],
                rhs=keys_bf[:, sc, 0:512],
                start=(first and sc == 0),
                stop=(last and sc == SC - 1),
            )
            nc.tensor.matmul(
                ps_c1,
                lhsT=wT_sb[:, sc : sc + 1],
                rhs=keys_bf[:, sc, 512:1024],
                start=(first and sc == 0),
                stop=(last and sc == SC - 1),
            )
            nc.tensor.matmul(
                ps_z,
                lhsT=wT_sb[:, sc : sc + 1],
                rhs=ones_bf[:, 0:1],
                start=(first and sc == 0),
                stop=(last and sc == SC - 1),
            )
        if last:
            finalize_batch(b, ps_c0, ps_c1, ctx_psums[b][2])

    def finalize_batch(b, ps_c0, ps_c1, ps_z):
        # finalize batch: out = ctx / Z
        rz = smalls.tile([1, 1], F32, tag="rz")
        nc.vector.reciprocal(rz, ps_z)
        out_sb = smalls.tile([1, H], F32, tag="out")
        nc.vector.tensor_scalar_mul(out_sb[0:1, 0:512], ps_c0, rz)
        nc.vector.tensor_scalar_mul(out_sb[0:1, 512:1024], ps_c1, rz)
        nc.gpsimd.dma_start(out=d_out[b : b + 1, :], in_=out_sb)

    n = len(iters)
    W = 4
    for i in range(0, n + W, W):
        for j in range(i, i + W):
            if j < n:
                front_loads[iters[j]] = stage_load(*iters[j])
        for j in range(i, i + W):
            if j < n:
                front[iters[j]] = stage_xpose(*iters[j])
                front_loads.pop(iters[j])
        for j in range(i - W, i):
            if 0 <= j < n:
                stage_compute(*iters[j])


def d_wq_rearr(d_w):
    # (H, A) dram -> [h' (part), hc, a] view
    return d_w.rearrange("(hc p) a -> p hc a", p=P)
_CACHED_NC = None


def _get_nc():
    global _CACHED_NC
    if _CACHED_NC is None:
        _CACHED_NC = build_bass()
    return _CACHED_NC


def kernel(query, keys, Wq, Wk, v):
    query = np.ascontiguousarray(np.asarray(query, dtype=np.float32))
    keys = np.ascontiguousarray(np.asarray(keys, dtype=np.float32))
    Wq = np.ascontiguousarray(np.asarray(Wq, dtype=np.float32))
    Wk = np.ascontiguousarray(np.asarray(Wk, dtype=np.float32))
    v = np.ascontiguousarray(np.asarray(v, dtype=np.float32))

    nc = _get_nc()
    in_maps = []
    for c in range(N_CORES):
        sl = slice(c * B_LOC, (c + 1) * B_LOC)
        in_maps.append(
            {
                "query": query[sl],
                "keys": keys[sl],
                "Wq": Wq,
                "Wk": Wk,
                "v": v,
            }
        )
    res = run_bass_kernel_spmd(nc, in_maps, list(range(N_CORES)))
    out = np.concatenate([res.results[c]["out"] for c in range(N_CORES)], axis=0)
    return out.reshape(B, 1, H).astype(np.float32)


if __name__ == "__main__":
    rng = np.random.default_rng(0)
    q = rng.standard_normal((B, H), dtype=np.float32)
    k = rng.standard_normal((B, S, H), dtype=np.float32)
    wq = rng.standard_normal((H, A), dtype=np.float32) / np.sqrt(H)
    wk = rng.standard_normal((H, A), dtype=np.float32) / np.sqrt(H)
    vv = rng.standard_normal((A,), dtype=np.float32) / np.sqrt(A)
    o = kernel(query=q, keys=k, Wq=wq, Wk=wk, v=vv)
    print(o.shape, o.dtype)


# revision 29
# speedup vs baseline: 1.0377x; 1.0377x over previous
"""Bahdanau temporal attention on 8 Trainium2 NeuronCores.

Full-input contract: kernel(**inputs) takes the unsharded numpy arrays
(query (32,1024), keys (32,4096,1024), Wq (1024,512), Wk (1024,512),
v (512,)) and returns the full output (32,1,1024) float32.

Sharding: data-parallel over batch. Each of the 8 cores processes 4
batches; Wq/Wk/v are replicated. No collectives.

Per-core algorithm (B_loc=4, S=4096, H=1024, A=512):
  q_t      = query @ Wq                          (B_loc, A)
  k_t      = keys @ Wk                           (B_loc, S, A)
  energy   = v . tanh(q_t + k_t)                 (B_loc, S)
  w        = exp(energy)         (unnormalized; |energy| <= |v|_1 so no
                                  max-subtraction is needed in fp32)
  ctx      = (w @ keys) / sum(w)                 (B_loc, H)

Single pass over keys. Per 512-row S-tile:
  - DMA keys tile (f32), DVE-cast to bf16,
  - xbar DMA-transpose bf16 -> keysT (contraction dim on partitions),
  - PE: k_t^T = Wk^T @ keys^T (bf16, f32 accum in PSUM),
  - ACT: T = tanh(k_t^T + q_t^T) with q_t as per-partition bias,
  - PE: energy = v^T @ T (fp32r),
  - ACT: w = exp(energy) with fused running-sum (accum_out),
  - PE transpose w to partitions; PE: ctx += w^T.T @ keys_nat (fp32r,
    full-f32 keys for accuracy).
"""

import sys

if "/opt/trn_rl_repo" not in sys.path:
    sys.path.insert(0, "/opt/trn_rl_repo")

import numpy as np

import concourse.bass as bass
import concourse.tile as tile
from concourse import bacc
from concourse import mybir
from concourse.bass_utils import run_bass_kernel_spmd
from concourse.masks import make_identity

F32 = mybir.dt.float32
F32R = mybir.dt.float32r
BF16 = mybir.dt.bfloat16

N_CORES = 8
B, S, H, A = 32, 4096, 1024, 512
B_LOC = B // N_CORES          # 4 batches per core
ST = 512                      # S-tile rows
N_ST = S // ST                # 8 S-tiles per batch
P = 128                       # partitions
HC = H // P                   # 8 contraction chunks
AC = A // P                   # 4 a-chunks
SC = ST // P                  # 4 s-chunks per S-tile


def build_bass():
    nc = bacc.Bacc()

    d_query = nc.declare_dram_parameter("query", [B_LOC, H], F32, isOutput=False)
    d_keys = nc.declare_dram_parameter("keys", [B_LOC, S, H], F32, isOutput=False)
    d_wq = nc.declare_dram_parameter("Wq", [H, A], F32, isOutput=False)
    d_wk = nc.declare_dram_parameter("Wk", [H, A], F32, isOutput=False)
    d_v = nc.declare_dram_parameter("v", [A], F32, isOutput=False)
    d_out = nc.declare_dram_parameter("out", [B_LOC, H], F32, isOutput=True)

    from contextlib import ExitStack

    with tile.TileContext(nc) as tc, ExitStack() as ctx:
        build_kernel_body(tc, d_query, d_keys, d_wq, d_wk, d_v, d_out, ctx)
    nc.compile()
    return nc


def build_kernel_body(tc, d_query, d_keys, d_wq, d_wk, d_v, d_out, ctx):
    nc = tc.nc

    consts = ctx.enter_context(tc.tile_pool(name="consts", bufs=1))
    keybf = ctx.enter_context(tc.tile_pool(name="keybf", bufs=6))
    keytp = ctx.enter_context(tc.tile_pool(name="keytp", bufs=6))
    tp = ctx.enter_context(tc.tile_pool(name="tp", bufs=3))
    smalls = ctx.enter_context(tc.tile_pool(name="smalls", bufs=2))
    pp_kt = ctx.enter_context(tc.tile_pool(name="pp_kt", bufs=3, space="PSUM"))
    pp_e = ctx.enter_context(tc.tile_pool(name="pp_e", bufs=2, space="PSUM"))
    pp_ctx = ctx.enter_context(tc.tile_pool(name="pp_ctx", bufs=3, space="PSUM"))

    # ---- constants ----
    # Wk in bf16, laid out [h' (part), hc, a]
    wk_bf = consts.tile([P, HC, A], BF16)
    nc.gpsimd.dma_start(
        out=wk_bf, in_=d_wq_rearr(d_wk)
    )  # SWDGE casts f32 -> bf16 in flight
    # Wq in bf16, same layout
    wq_sb = consts.tile([P, HC, A], BF16)
    nc.gpsimd.dma_start(out=wq_sb, in_=d_wq_rearr(d_wq))

    # v: load f32, DVE-cast into row 0 of a 16-row tile (single-producer
    # funnel so the xbar transpose carries only one wait), then xbar.
    v_f32 = consts.tile([1, A], F32)
    nc.gpsimd.dma_start(out=v_f32, in_=d_v[None, :])
    v16 = consts.tile([16, A], BF16)
    nc.vector.memset(v16, 0.0)
    nc.vector.tensor_copy(v16[0:1, :], v_f32)
    vT16 = consts.tile([P, AC, 16], BF16)
    nc.sync.dma_start(out=vT16, in_=v16, transpose=True)

    # query: same funnel pattern
    q_f32 = consts.tile([B_LOC, H], F32)
    nc.gpsimd.dma_start(out=q_f32, in_=d_query[:, :])
    q16 = consts.tile([16, H], BF16)
    nc.vector.memset(q16, 0.0)
    nc.vector.tensor_copy(q16[0:B_LOC, :], q_f32)
    qT16 = consts.tile([P, HC, 16], BF16)
    nc.sync.dma_start(out=qT16, in_=q16, transpose=True)

    # q_t = query @ Wq : psum (16, A), accumulate over hc
    ps_qt = pp_e.tile([16, A], F32, tag="pe")
    for hc in range(HC):
        nc.tensor.matmul(
            ps_qt,
            lhsT=qT16[:, hc, :],
            rhs=wq_sb[:, hc, :],
            start=(hc == 0),
            stop=(hc == HC - 1),
        )
    qt16 = consts.tile([16, A], BF16)
    nc.vector.memset(qt16, 0.0)
    nc.vector.tensor_copy(qt16[0:B_LOC, :], ps_qt[0:B_LOC, :])
    # xbar -> qtT16 (128, AC, 16); tanh bias per (ac, b) = qtT16[:, ac, b]
    qtT16 = consts.tile([P, AC, 16], BF16)
    nc.sync.dma_start(out=qtT16, in_=qt16, transpose=True)

    ones_bf = consts.tile([P, 1], BF16)
    nc.vector.memset(ones_bf, 1.0)

    # ---- main loop (2-stage pipelined emission: front i, compute i-1) ----
    iters = [(b, st) for b in range(B_LOC) for st in range(N_ST)]
    ctx_psums = {}
    front = {}
    front_loads = {}

    def stage_load(b, st):
        # load keys tile natural [s' (part), r, h], f32 -> bf16 in the DMA
        keys_bf = keybf.tile([P, SC, H], BF16, tag="kbf")
        nc.gpsimd.dma_start(
            out=keys_bf,
            in_=d_keys[b, st * ST : (st + 1) * ST, :].rearrange(
                "(p r) h -> p r h", p=P
            ),
        )
        return keys_bf

    def stage_xpose(b, st):
        keys_bf = front_loads[(b, st)]
        # transpose: keysT [h' (part), sc, hc, s']
        keysT = keytp.tile([P, SC, HC, P], BF16, tag="kT")
        for j in range(2):
            nc.sync.dma_start(
                out=keysT[:, 2 * j : 2 * j + 2, :, :],
                in_=keys_bf[:, 2 * j : 2 * j + 2, :],
                transpose=True,
            )
        return keys_bf, keysT

    def stage_compute(b, st):
        keys_bf, keysT = front.pop((b, st))

        first = st == 0
        last = st == N_ST - 1
        if first:
            ps_c0_new = pp_ctx.tile([1, 512], F32, tag="ctx")
            ps_c1_new = pp_ctx.tile([1, 512], F32, tag="ctx")
            ps_z_new = pp_ctx.tile([1, 1], F32, tag="ctx")
            ctx_psums[b] = (ps_c0_new, ps_c1_new, ps_z_new)
        ps_c0, ps_c1, _ = ctx_psums[b]

        # projection + tanh: T[a' (part), ac, s]
        T_sb = tp.tile([P, AC, ST], BF16, tag="T")
        for ac in range(AC):
            ps_kt = pp_kt.tile([P, ST], F32, tag="kt")
            for hc in range(HC):
                nc.tensor.matmul(
                    ps_kt,
                    lhsT=wk_bf[:, hc, ac * P : (ac + 1) * P],
                    rhs=keysT[:, :, hc, :],
                    start=(hc == 0),
                    stop=(hc == HC - 1),
                )
            nc.scalar.activation(
                T_sb[:, ac, :],
                ps_kt,
                mybir.ActivationFunctionType.Tanh,
                bias=qtT16[:, ac, b : b + 1],
            )

        # energy transposed: eT (128, SC) via regular matmuls (M=s chunk)
        ps_eT = pp_e.tile([P, SC], F32, tag="pe")
        for sc in range(SC):
            for ac in range(AC):
                nc.tensor.matmul(
                    ps_eT[:, sc : sc + 1],
                    lhsT=T_sb[:, ac, sc * P : (sc + 1) * P],
                    rhs=vT16[:, ac, 0:1],
                    start=(ac == 0),
                    stop=(ac == AC - 1),
                )

        # w^T = exp(eT) straight into SBUF, already s-on-partitions
        wT_sb = smalls.tile([P, SC], BF16, tag="wT")
        nc.scalar.activation(
            wT_sb,
            ps_eT,
            mybir.ActivationFunctionType.Exp,
        )

        # context accumulation: ctx (1, H) += w^T.T @ keys_bf
        # plus Z accumulation with a ones column (same bf16 weights as ctx)
        ps_z = ctx_psums[b][2]
        for sc in range(SC):
            nc.tensor.matmul(
                ps_c0,
                lhsT=wT_sb[:, sc : sc + 1],
                rhs=keys_bf[:, sc, 0:512],
                start=(first and sc == 0),
                stop=(last and sc == SC - 1),
            )
            nc.tensor.matmul(
                ps_c1,
                lhsT=wT_sb[:, sc : sc + 1],
                rhs=keys_bf[:, sc, 512:1024],
                start=(first and sc == 0),
                stop=(last and sc == SC - 1),
            )
            nc.tensor.matmul(
                ps_z,
                lhsT=wT_sb[:, sc : sc + 1],
                rhs=ones_bf[:, 0:1],
                start=(first and sc == 0),
                stop=(last and sc == SC - 1),
            )
        if last:
            finalize_batch(b, ps_c0, ps_c1, ctx_psums[b][2])

    def finalize_batch(b, ps_c0, ps_c1, ps_z):
        # finalize batch: out = ctx / Z
        rz = smalls.tile([1, 1], F32, tag="rz")
        nc.vector.reciprocal(rz, ps_z)
        out_sb = smalls.tile([1, H], F32, tag="out")
        nc.vector.tensor_scalar_mul(out_sb[0:1, 0:512], ps_c0, rz)
        nc.vector.tensor_scalar_mul(out_sb[0:1, 512:1024], ps_c1, rz)
        nc.gpsimd.dma_start(out=d_out[b : b + 1, :], in_=out_sb)

    n = len(iters)
    for i in range(n + 1):
        if i < n:
            front_loads[iters[i]] = stage_load(*iters[i])
            front[iters[i]] = stage_xpose(*iters[i])
            front_loads.pop(iters[i])
        if i >= 1:
            stage_compute(*iters[i - 1])


def d_wq_rearr(d_w):
    # (H, A) dram -> [h' (part), hc, a] view
    return d_w.rearrange("(hc p) a -> p hc a", p=P)
_CACHED_NC = None


def _get_nc():
    global _CACHED_NC
    if _CACHED_NC is None:
        _CACHED_NC = build_bass()
    return _CACHED_NC


def kernel(query, keys, Wq, Wk, v):
    query = np.ascontiguousarray(np.asarray(query, dtype=np.float32))
    keys = np.ascontiguousarray(np.asarray(keys, dtype=np.float32))
    Wq = np.ascontiguousarray(np.asarray(Wq, dtype=np.float32))
    Wk = np.ascontiguousarray(np.asarray(Wk, dtype=np.float32))
    v = np.ascontiguousarray(np.asarray(v, dtype=np.float32))

    nc = _get_nc()
    in_maps = []
    for c in range(N_CORES):
        sl = slice(c * B_LOC, (c + 1) * B_LOC)
        in_maps.append(
            {
                "query": query[sl],
                "keys": keys[sl],
                "Wq": Wq,
                "Wk": Wk,
                "v": v,
            }
        )
    res = run_bass_kernel_spmd(nc, in_maps, list(range(N_CORES)))
    out = np.concatenate([res.results[c]["out"] for c in range(N_CORES)], axis=0)
    return out.reshape(B, 1, H).astype(np.float32)


if __name__ == "__main__":
    rng = np.random.default_rng(0)
    q = rng.standard_normal((B, H), dtype=np.float32)
    k = rng.standard_normal((B, S, H), dtype=np.float32)
    wq = rng.standard_normal((H, A), dtype=np.float32) / np.sqrt(H)
    wk = rng.standard_normal((H, A), dtype=np.float32) / np.sqrt(H)
    vv = rng.standard_normal((A,), dtype=np.float32) / np.sqrt(A)
    o = kernel(query=q, keys=k, Wq=wq, Wk=wk, v=vv)
    print(o.shape, o.dtype)


# revision 30
# speedup vs baseline: 1.0556x; 1.0173x over previous
"""Bahdanau temporal attention on 8 Trainium2 NeuronCores.

Full-input contract: kernel(**inputs) takes the unsharded numpy arrays
(query (32,1024), keys (32,4096,1024), Wq (1024,512), Wk (1024,512),
v (512,)) and returns the full output (32,1,1024) float32.

Sharding: data-parallel over batch. Each of the 8 cores processes 4
batches; Wq/Wk/v are replicated. No collectives.

Per-core algorithm (B_loc=4, S=4096, H=1024, A=512):
  q_t      = query @ Wq                          (B_loc, A)
  k_t      = keys @ Wk                           (B_loc, S, A)
  energy   = v . tanh(q_t + k_t)                 (B_loc, S)
  w        = exp(energy)         (unnormalized; |energy| <= |v|_1 so no
                                  max-subtraction is needed in fp32)
  ctx      = (w @ keys) / sum(w)                 (B_loc, H)

Single pass over keys. Per 512-row S-tile:
  - DMA keys tile (f32), DVE-cast to bf16,
  - xbar DMA-transpose bf16 -> keysT (contraction dim on partitions),
  - PE: k_t^T = Wk^T @ keys^T (bf16, f32 accum in PSUM),
  - ACT: T = tanh(k_t^T + q_t^T) with q_t as per-partition bias,
  - PE: energy = v^T @ T (fp32r),
  - ACT: w = exp(energy) with fused running-sum (accum_out),
  - PE transpose w to partitions; PE: ctx += w^T.T @ keys_nat (fp32r,
    full-f32 keys for accuracy).
"""

import sys

if "/opt/trn_rl_repo" not in sys.path:
    sys.path.insert(0, "/opt/trn_rl_repo")

import numpy as np

import concourse.bass as bass
import concourse.tile as tile
from concourse import bacc
from concourse import mybir
from concourse.bass_utils import run_bass_kernel_spmd
from concourse.masks import make_identity

F32 = mybir.dt.float32
F32R = mybir.dt.float32r
BF16 = mybir.dt.bfloat16

N_CORES = 8
B, S, H, A = 32, 4096, 1024, 512
B_LOC = B // N_CORES          # 4 batches per core
ST = 512                      # S-tile rows
N_ST = S // ST                # 8 S-tiles per batch
P = 128                       # partitions
HC = H // P                   # 8 contraction chunks
AC = A // P                   # 4 a-chunks
SC = ST // P                  # 4 s-chunks per S-tile


def build_bass():
    nc = bacc.Bacc()

    d_query = nc.declare_dram_parameter("query", [B_LOC, H], F32, isOutput=False)
    d_keys = nc.declare_dram_parameter("keys", [B_LOC, S, H], F32, isOutput=False)
    d_wq = nc.declare_dram_parameter("Wq", [H, A], F32, isOutput=False)
    d_wk = nc.declare_dram_parameter("Wk", [H, A], F32, isOutput=False)
    d_v = nc.declare_dram_parameter("v", [A], F32, isOutput=False)
    d_out = nc.declare_dram_parameter("out", [B_LOC, H], F32, isOutput=True)

    from contextlib import ExitStack

    with tile.TileContext(nc) as tc, ExitStack() as ctx:
        build_kernel_body(tc, d_query, d_keys, d_wq, d_wk, d_v, d_out, ctx)
    nc.compile()
    return nc


def build_kernel_body(tc, d_query, d_keys, d_wq, d_wk, d_v, d_out, ctx):
    nc = tc.nc

    consts = ctx.enter_context(tc.tile_pool(name="consts", bufs=1))
    keyp = ctx.enter_context(tc.tile_pool(name="keyp", bufs=4))
    keybf = ctx.enter_context(tc.tile_pool(name="keybf", bufs=4))
    keytp = ctx.enter_context(tc.tile_pool(name="keytp", bufs=4))
    tp = ctx.enter_context(tc.tile_pool(name="tp", bufs=3))
    smalls = ctx.enter_context(tc.tile_pool(name="smalls", bufs=4))
    pp_kt = ctx.enter_context(tc.tile_pool(name="pp_kt", bufs=3, space="PSUM"))
    pp_e = ctx.enter_context(tc.tile_pool(name="pp_e", bufs=2, space="PSUM"))
    pp_ctx = ctx.enter_context(tc.tile_pool(name="pp_ctx", bufs=3, space="PSUM"))

    # ---- constants ----
    # Wk in bf16, laid out [h' (part), hc, a]
    wk_bf = consts.tile([P, HC, A], BF16)
    nc.gpsimd.dma_start(
        out=wk_bf, in_=d_wq_rearr(d_wk)
    )  # SWDGE casts f32 -> bf16 in flight
    # Wq in bf16, same layout
    wq_sb = consts.tile([P, HC, A], BF16)
    nc.gpsimd.dma_start(out=wq_sb, in_=d_wq_rearr(d_wq))

    # v: load f32, DVE-cast into row 0 of a 16-row tile (single-producer
    # funnel so the xbar transpose carries only one wait), then xbar.
    v_f32 = consts.tile([1, A], F32)
    nc.gpsimd.dma_start(out=v_f32, in_=d_v[None, :])
    v16 = consts.tile([16, A], BF16)
    nc.vector.memset(v16, 0.0)
    nc.vector.tensor_copy(v16[0:1, :], v_f32)
    vT16 = consts.tile([P, AC, 16], BF16)
    nc.sync.dma_start(out=vT16, in_=v16, transpose=True)

    # query: same funnel pattern
    q_f32 = consts.tile([B_LOC, H], F32)
    nc.gpsimd.dma_start(out=q_f32, in_=d_query[:, :])
    q16 = consts.tile([16, H], BF16)
    nc.vector.memset(q16, 0.0)
    nc.vector.tensor_copy(q16[0:B_LOC, :], q_f32)
    qT16 = consts.tile([P, HC, 16], BF16)
    nc.sync.dma_start(out=qT16, in_=q16, transpose=True)

    # q_t = query @ Wq : psum (16, A), accumulate over hc
    ps_qt = pp_e.tile([16, A], F32, tag="pe")
    for hc in range(HC):
        nc.tensor.matmul(
            ps_qt,
            lhsT=qT16[:, hc, :],
            rhs=wq_sb[:, hc, :],
            start=(hc == 0),
            stop=(hc == HC - 1),
        )
    qt16 = consts.tile([16, A], BF16)
    nc.vector.memset(qt16, 0.0)
    nc.vector.tensor_copy(qt16[0:B_LOC, :], ps_qt[0:B_LOC, :])
    # xbar -> qtT16 (128, AC, 16); tanh bias per (ac, b) = qtT16[:, ac, b]
    qtT16 = consts.tile([P, AC, 16], BF16)
    nc.sync.dma_start(out=qtT16, in_=qt16, transpose=True)

    ones_bf = consts.tile([P, 1], BF16)
    nc.vector.memset(ones_bf, 1.0)

    # ---- main loop (2-stage pipelined emission: front i, compute i-1) ----
    iters = [(b, st) for b in range(B_LOC) for st in range(N_ST)]
    ctx_psums = {}
    front = {}
    front_loads = {}

    def stage_load(b, st):
        # load keys tile natural [s' (part), r, h] f32, then DVE-cast to bf16
        keys_nat = keyp.tile([P, SC, H], F32, tag="keys")
        nc.scalar.dma_start(
            out=keys_nat,
            in_=d_keys[b, st * ST : (st + 1) * ST, :].rearrange(
                "(p r) h -> p r h", p=P
            ),
        )
        keys_bf = keybf.tile([P, SC, H], BF16, tag="kbf")
        nc.vector.tensor_copy(keys_bf, keys_nat)
        return keys_bf

    def stage_xpose(b, st):
        keys_bf = front_loads[(b, st)]
        # transpose: keysT [h' (part), sc, hc, s']
        keysT = keytp.tile([P, SC, HC, P], BF16, tag="kT")
        for j in range(2):
            nc.sync.dma_start(
                out=keysT[:, 2 * j : 2 * j + 2, :, :],
                in_=keys_bf[:, 2 * j : 2 * j + 2, :],
                transpose=True,
            )
        return keys_bf, keysT

    def stage_compute(b, st):
        keys_bf, keysT = front.pop((b, st))

        first = st == 0
        last = st == N_ST - 1
        if first:
            ps_c0_new = pp_ctx.tile([1, 512], F32, tag="ctx")
            ps_c1_new = pp_ctx.tile([1, 512], F32, tag="ctx")
            ps_z_new = pp_ctx.tile([1, 1], F32, tag="ctx")
            ctx_psums[b] = (ps_c0_new, ps_c1_new, ps_z_new)
        ps_c0, ps_c1, _ = ctx_psums[b]

        # projection + tanh: T[a' (part), ac, s]
        T_sb = tp.tile([P, AC, ST], BF16, tag="T")
        for ac in range(AC):
            ps_kt = pp_kt.tile([P, ST], F32, tag="kt")
            for hc in range(HC):
                nc.tensor.matmul(
                    ps_kt,
                    lhsT=wk_bf[:, hc, ac * P : (ac + 1) * P],
                    rhs=keysT[:, :, hc, :],
                    start=(hc == 0),
                    stop=(hc == HC - 1),
                )
            nc.scalar.activation(
                T_sb[:, ac, :],
                ps_kt,
                mybir.ActivationFunctionType.Tanh,
                bias=qtT16[:, ac, b : b + 1],
            )

        # energy transposed: eT (128, SC) via regular matmuls (M=s chunk)
        ps_eT = pp_e.tile([P, SC], F32, tag="pe")
        for sc in range(SC):
            for ac in range(AC):
                nc.tensor.matmul(
                    ps_eT[:, sc : sc + 1],
                    lhsT=T_sb[:, ac, sc * P : (sc + 1) * P],
                    rhs=vT16[:, ac, 0:1],
                    start=(ac == 0),
                    stop=(ac == AC - 1),
                )

        # w^T = exp(eT) straight into SBUF, already s-on-partitions
        wT_sb = smalls.tile([P, SC], BF16, tag="wT")
        nc.scalar.activation(
            wT_sb,
            ps_eT,
            mybir.ActivationFunctionType.Exp,
        )

        # context accumulation: ctx (1, H) += w^T.T @ keys_bf
        # plus Z accumulation with a ones column (same bf16 weights as ctx)
        ps_z = ctx_psums[b][2]
        for sc in range(SC):
            nc.tensor.matmul(
                ps_c0,
                lhsT=wT_sb[:, sc : sc + 1],
                rhs=keys_bf[:, sc, 0:512],
                start=(first and sc == 0),
                stop=(last and sc == SC - 1),
            )
            nc.tensor.matmul(
                ps_c1,
                lhsT=wT_sb[:, sc : sc + 1],
                rhs=keys_bf[:, sc, 512:1024],
                start=(first and sc == 0),
                stop=(last and sc == SC - 1),
            )
            nc.tensor.matmul(
                ps_z,
                lhsT=wT_sb[:, sc : sc + 1],
                rhs=ones_bf[:, 0:1],
                start=(first and sc == 0),
                stop=(last and sc == SC - 1),
            )
        if last:
            finalize_batch(b, ps_c0, ps_c1, ctx_psums[b][2])

    def finalize_batch(b, ps_c0, ps_c1, ps_z):
        # finalize batch: out = ctx / Z
        rz = smalls.tile([1, 1], F32, tag="rz")
        nc.vector.reciprocal(rz, ps_z)
        out_sb = smalls.tile([1, H], F32, tag="out")
        nc.vector.tensor_scalar_mul(out_sb[0:1, 0:512], ps_c0, rz)
        nc.vector.tensor_scalar_mul(out_sb[0:1, 512:1024], ps_c1, rz)
        nc.gpsimd.dma_start(out=d_out[b : b + 1, :], in_=out_sb)

    n = len(iters)
    for i in range(n + 1):
        if i < n:
            front_loads[iters[i]] = stage_load(*iters[i])
            front[iters[i]] = stage_xpose(*iters[i])
            front_loads.pop(iters[i])
        if i >= 1:
            stage_compute(*iters[i - 1])


def d_wq_rearr(d_w):
    # (H, A) dram -> [h' (part), hc, a] view
    return d_w.rearrange("(hc p) a -> p hc a", p=P)
_CACHED_NC = None


def _get_nc():
    global _CACHED_NC
    if _CACHED_NC is None:
        _CACHED_NC = build_bass()
    return _CACHED_NC


def kernel(query, keys, Wq, Wk, v):
    query = np.ascontiguousarray(np.asarray(query, dtype=np.float32))
    keys = np.ascontiguousarray(np.asarray(keys, dtype=np.float32))
    Wq = np.ascontiguousarray(np.asarray(Wq, dtype=np.float32))
    Wk = np.ascontiguousarray(np.asarray(Wk, dtype=np.float32))
    v = np.ascontiguousarray(np.asarray(v, dtype=np.float32))

    nc = _get_nc()
    in_maps = []
    for c in range(N_CORES):
        sl = slice(c * B_LOC, (c + 1) * B_LOC)
        in_maps.append(
            {
                "query": query[sl],
                "keys": keys[sl],
                "Wq": Wq,
                "Wk": Wk,
                "v": v,
            }
        )
    res = run_bass_kernel_spmd(nc, in_maps, list(range(N_CORES)))
    out = np.concatenate([res.results[c]["out"] for c in range(N_CORES)], axis=0)
    return out.reshape(B, 1, H).astype(np.float32)


if __name__ == "__main__":
    rng = np.random.default_rng(0)
    q = rng.standard_normal((B, H), dtype=np.float32)
    k = rng.standard_normal((B, S, H), dtype=np.float32)
    wq = rng.standard_normal((H, A), dtype=np.float32) / np.sqrt(H)
    wk = rng.standard_normal((H, A), dtype=np.float32) / np.sqrt(H)
    vv = rng.standard_normal((A,), dtype=np.float32) / np.sqrt(A)
    o = kernel(query=q, keys=k, Wq=wq, Wk=wk, v=vv)
    print(o.shape, o.dtype)
